# revision 22
# baseline (speedup 1.0000x reference)
"""RNN-T decoder kernel for TRN2 (8 cores, T-sharded joint, replicated LSTM).

v3: chain-split + chunked prologue.

Layout notes
------------
B=8, T=128, U=64, E=512, H=1024 (8 k-chunks), J=640 (5 j-chunks), OD=1024.
Each core handles T-slice [16c, 16c+16) of the joint; the 2-layer LSTM over U
is computed identically (replicated, all 8 batches) on every core.

Gate permutation: hidden dim is split in 4 quarters (col-tile groups). Group
j's 1024 gate columns are [i_j | g_j | f_j | o_j] (256 each), where x_j acts
on hidden units [256j, 256j+256). Weights/bias/X tensors are host-permuted
to this order.  hf0 = [i|g] (psum bank A), hf1 = [f|o] (bank B): the i/g
activations + t1 mul run while the hf1 matmul packs stream, halving the
post-matmul serial chain.

Gates PSUM tile (128, 1024): group j occupies partitions [32j, 32j+8)
(batch-major), accumulated by 4-way column-packed matmuls (tile_position).

v3 changes vs v2:
- hf-split recurrent packs (hf-outer loop) with per-hf inject; i/g ACTs and
  the t1 DVE mul overlap the hf1 packs by PSUM-region dependency.
- Prologue weight DMAs chunked (per-kc / per-nc) across sync/scalar/gpsimd
  queues: the LSTM starts as soon as the first chunks land, not after 8MB.
- embed gathered in fp16 directly (host pre-cast) - no CAST on gather path.
- wdec SBUF-resident (was re-streamed 10MB); zd in 16-u pair chunks where
  availability allows.
- joint stage2 jc-outer (shared LDWEIGHTS, both PSUM bufs live).
- pending2 joint-filler cap 2 -> 3.
"""
import numpy as np
import ml_dtypes

import concourse.bass as bass
import concourse.bacc as bacc
import concourse.mybir as mybir
import concourse.tile as tile

dt = mybir.dt
AF = mybir.ActivationFunctionType

B, T, E, H, J, OD = 8, 128, 512, 1024, 640, 1024
HK = H // 128   # 8 h-chunks
JC = J // 128   # 5 j-chunks
EK = E // 128   # 4 e-chunks
TSH = T // 8    # 16 t per core
NG = 4          # col-tile groups

W1_RING = 2     # wih1 chunk ring depth (4KB/part each)
XG_BUFS = 2


def build_program(U=64, n_cores=8, with_biases=False,
                  with_out_bias=False):
    nc = bacc.Bacc("TRN2", target_bir_lowering=False, debug=False,
                   num_devices=n_cores)
    f16, f32, i32 = dt.float16, dt.float32, dt.int32
    f8 = dt.float8e4
    UG = U // 16  # u-blocks of 16
    assert U % 16 == 0

    # ---------------- external inputs ----------------
    embed_d = nc.dram_tensor("embedt", [128, OD // 128, E], f16,
                             kind="ExternalInput")
    yidxr_d = nc.dram_tensor("yidxr", [128, B * U], f32, kind="ExternalInput")
    slotv_d = nc.dram_tensor("slotv", [128, OD // 128], f32,
                             kind="ExternalInput")
    wih0_d = nc.dram_tensor("wih0t", [128, 8, EK, 512], f8, kind="ExternalInput")
    wih1_d = nc.dram_tensor("wih1t", [128, 16, HK, 256], f8, kind="ExternalInput")
    whh0_d = nc.dram_tensor("whh0t", [128, HK, NG, 1024], f8, kind="ExternalInput")
    whh1_d = nc.dram_tensor("whh1t", [128, HK, NG, 1024], f8, kind="ExternalInput")
    eye128_d = nc.dram_tensor("eye128", [128, 128], f16, kind="ExternalInput")
    injrep_d = nc.dram_tensor("injrep", [128, 4, 8], f16, kind="ExternalInput")
    wenc_d = nc.dram_tensor("wenct", [JC, 128, HK, 128], f16, kind="ExternalInput")
    wdec_d = nc.dram_tensor("wdect", [128, JC, HK, 128], f16, kind="ExternalInput")
    wout_d = nc.dram_tensor("woutt", [128, JC, OD], f16, kind="ExternalInput")
    benc_d = nc.dram_tensor("benc", [128, JC], f32, kind="ExternalInput")
    bout_d = nc.dram_tensor("boutrep", [128, OD], f32, kind="ExternalInput")
    hst_d = nc.dram_tensor("hst16", [128, HK, B * TSH], f16, kind="ExternalInput")
    # per-layer (b_ih + b_hh), gate-permuted, replicated over partitions
    bi0_d = nc.dram_tensor("bihh0", [128, 4096], f16, kind="ExternalInput")
    bi1_d = nc.dram_tensor("bihh1", [128, 4096], f16, kind="ExternalInput")

    out_d = nc.dram_tensor("out", [B * TSH * U, OD], f16, kind="ExternalOutput")

    # ---------------- internal dram ----------------
    # grouped X quad layout: [u//4, group j, row r = 8*(u%4)+b, 1024]
    x0_d = nc.dram_tensor("X0d", [U // 4, NG, 32, 1024], f16)
    x1_d = nc.dram_tensor("X1d", [U // 4, NG, 32, 1024], f16)

    with tile.TileContext(nc) as tc:
        with (
            tc.tile_pool(name="const", bufs=1) as pc,
            tc.tile_pool(name="lstmS", bufs=1) as lS,
            tc.tile_pool(name="lstmPS", bufs=1, space="PSUM") as lP,
        ):
            # h_dec transposed history, both layers (fp16)
            hdec = [pc.tile([128, HK, U, B], f16, tag=f"hdec{l}",
                            name=f"hdec{l}") for l in range(2)]
            if with_biases:
                bi0_sb = pc.tile([128, 4096], f16, tag="bi0")
                nc.gpsimd.dma_start(bi0_sb[:], bi0_d.ap())
                bi1_sb = pc.tile([128, 4096], f16, tag="bi1")
                nc.gpsimd.dma_start(bi1_sb[:], bi1_d.ap())

            # recurrent weights (resident for whole LSTM); whh1 gets its own
            # pool opened at w8 (after the prologue pool frees its space)
            whh_sb = [pc.tile([128, HK, NG, 1024], f8, tag="whh0",
                              name="whh0"), None]

            # gather consts first (prologue critical path)
            yidxr_sb = pc.tile([128, B * U], f32, tag="yidxr")
            nc.sync.dma_start(yidxr_sb[:], yidxr_d.ap())
            slotv_sb = pc.tile([128, OD // 128], f32, tag="slotv")
            nc.sync.dma_start(slotv_sb[:], slotv_d.ap())
            # joint resident tiles (DMAs for wout emitted later)
            wout_sb = pc.tile([128, JC, OD], f16, tag="wout")
            ze_sb = pc.tile([128, JC, B * TSH], f16, tag="ze")
            wdec_sb = pc.tile([128, JC, HK, 128], f16, tag="wdecs")
            benc_sb = pc.tile([128, JC], f32, tag="bencs")
            nc.sync.dma_start(benc_sb[:], benc_d.ap())
            if with_out_bias:
                bout_sb = pc.tile([128, OD], f32, tag="bouts")
                nc.sync.dma_start(bout_sb[:], bout_d.ap())
            eye128_sb = pc.tile([128, 128], f16, tag="eye128")
            nc.sync.dma_start(eye128_sb[:], eye128_d.ap())
            injrep_sb = pc.tile([128, 4, 8], f16, tag="injrep")
            nc.sync.dma_start(injrep_sb[:], injrep_d.ap())

            # ---------------- prologue pool (released at w8) ----------------
            # yidx on the fast sync queue; the embedding gather's indirect
            # DMA must be at the head of the gpsimd queue so the prologue
            # critical path (gather -> X0 block 0 -> w00) starts promptly
            NCH = B * U // 128  # row chunks of 128 (one per u-block of 16)
            pre_ctx = tc.tile_pool(name="preS", bufs=1)
            pS = pre_ctx.__enter__()
            hst_sb = pc.tile([128, HK, B * TSH], f16, tag="hst")
            embed_sb = pS.tile([128, OD // 128, E], f16, tag="embed")
            nc.sync.dma_start(embed_sb[:, 0:4], embed_d.ap()[:, 0:4])
            nc.sync.dma_start(embed_sb[:, 4:8], embed_d.ap()[:, 4:8])
            wih0_sb = pS.tile([128, 8, EK, 512], f8, tag="wih0")
            # wih0 chunked per nc-pair so x0_block(0) starts after ~1MB;
            # whh0 chunked per kc so w01's first packs start after ~1MB.
            # Queues: scalar carries wih0, sync+gpsimd carry whh0.
            for c in range(4):
                nc.scalar.dma_start(wih0_sb[:, 2 * c:2 * c + 2],
                                    wih0_d.ap()[:, 2 * c:2 * c + 2])
            eyst = pS.tile([128, EK, B * U], f16, tag="eyst")

            NS = OD // 128  # embed-table slots

            def gather_chunk(ch):
                # one-hot matmul gather: eyst[:, :, ch] = sum_s
                # embed[s].T @ onehot[s]; replaces the slow SWDGE
                # indirect gather and the PE transposes
                oh = pS.tile([128, NS, 128], f16, tag="g16", bufs=2)
                yb = yidxr_sb[:, ch * 128:(ch + 1) * 128] \
                    .unsqueeze(1).to_broadcast((128, NS, 128))
                sv = slotv_sb[:].unsqueeze(2).to_broadcast((128, NS, 128))
                nc.vector.tensor_tensor(oh[:], sv, yb,
                                        op=mybir.AluOpType.is_equal)
                for ec in range(EK):
                    tp = lP.tile([128, 512], f32, tag="outps", bufs=2)
                    for s in range(NS):
                        nc.tensor.matmul(
                            tp[:, 0:128],
                            embed_sb[:, s, ec * 128:(ec + 1) * 128],
                            oh[:, s, :],
                            start=(s == 0), stop=(s == NS - 1))
                    nc.vector.tensor_copy(
                        eyst[:, ec, ch * 128:(ch + 1) * 128], tp[:, 0:128])

            def x0_block(g):
                # rows = (16 u, 8 b) of u-block g; full 4096 gate cols
                for j0 in range(NG):
                    x0cw = lS.tile([128, 1024], f16, tag="x1c", bufs=2)
                    for q in range(2):
                        nc_ = j0 * 2 + q
                        ps = lP.tile([128, 512], f32, tag="outps", bufs=2)
                        for ec in range(EK):
                            nc.tensor.matmul(
                                ps[:],
                                eyst[:, ec, g * 128:(g + 1) * 128],
                                wih0_sb[:, nc_, ec, :],
                                start=(ec == 0), stop=(ec == EK - 1))
                        sl = slice(q * 512, (q + 1) * 512)
                        if with_biases:
                            nc.vector.tensor_add(
                                x0cw[:, sl], ps[:],
                                bi0_sb[:, nc_ * 512:(nc_ + 1) * 512])
                        else:
                            nc.vector.tensor_copy(x0cw[:, sl], ps[:])
                    nc.gpsimd.dma_start(
                        x0_d.ap()[g * 4:(g + 1) * 4, j0, :, :],
                        x0cw[:])

            def ze_jc(jc):
                wec = lS.tile([128, HK, 128], f16, tag="wdc", bufs=2)
                nc.sync.dma_start(wec[:], wenc_d.ap()[jc])
                zp = lP.tile([128, 512], f32, tag="mmps", bufs=2)
                for ec in range(HK):
                    nc.tensor.matmul(zp[:, 0:128], wec[:, ec, :],
                                     hst_sb[:, ec, :],
                                     start=(ec == 0), stop=(ec == HK - 1))
                nc.scalar.activation(ze_sb[:, jc, :], zp[:, 0:128],
                                     AF.Identity,
                                     bias=benc_sb[:, jc:jc + 1])

            # ---------------- LSTM pieces ----------------
            gate_ps = [lP.tile([128, 1024], f32, tag=f"gates{l}",
                               name=f"gates{l}") for l in range(2)]
            czero = [lS.tile([128, 256], f32, tag=f"c{l}", name=f"cz{l}",
                             bufs=2) for l in range(2)]
            nc.gpsimd.memset(czero[0][:], 0.0)
            nc.gpsimd.memset(czero[1][:], 0.0)
            cprev = [czero[0], czero[1]]
            xsrc = [x0_d, x1_d]

            xq_pre = {}
            wih1_box = [None]

            def xq_fetch(l, q):
                # one 256KB quad fetch covers X for u in [4q, 4q+4); issued
                # >=1 wavefront ahead so the inject never stalls the PE
                # mid-burst (HAM stays warm)
                xq = lS.tile([128, 1024], f16, tag=f"xq{l}", bufs=XG_BUFS)
                nc.scalar.dma_start(xq[:], xsrc[l].ap()[q])
                xq_pre[(l, q)] = xq

            def lstm_mms(l, u):
                """hf-split matmul phase: packs+inject hf0, then the i/g
                acts + t1 (overlap the hf1 packs via PSUM-region deps),
                then packs+inject hf1.  The chain tail is emitted
                separately AFTER the wavefront's PE fillers so the static
                per-engine order never head-of-line-blocks the Tensor
                queue on chain-gated work."""
                pg = gate_ps[l]
                xg = xq_pre[(l, u // 4)]
                uu = u % 4
                sgi = tg = t1 = None
                for hf in range(2):
                    sl = slice(hf * 512, (hf + 1) * 512)
                    if u > 0:
                        for kc in range(HK):
                            for j in range(NG):
                                nc.tensor.matmul(
                                    pg[32 * j:32 * j + 8, sl],
                                    hdec[l][:, kc, u - 1, :],
                                    whh_sb[l][:, kc, j,
                                              hf * 512:(hf + 1) * 512],
                                    tile_position=(0, 32 * j),
                                    start=(kc == 0), stop=False)
                    # x inject: diagonal-tiled eye-matmuls reading the
                    # grouped xg slice for each group straight from SBUF
                    # partitions 32j..32j+8 (ends each region's group)
                    for j in range(NG):
                        nc.tensor.matmul(
                            pg[32 * j:32 * j + 8, sl],
                            injrep_sb[32 * j:32 * j + 32, uu, :],
                            xg[32 * j:32 * j + 32, sl],
                            tile_position=(32 * j, 32 * j),
                            start=(u == 0), stop=True)
                    if hf == 0:
                        # i/g acts + t1: ready as soon as bank A's regions
                        # close; overlap the hf1 packs (bank B)
                        sgi = lS.tile([128, 256], f16, tag=f"sgi{l}")
                        nc.scalar.activation(sgi[:], pg[:, 0:256], AF.Sigmoid,
                                             scale=1 / 64.0)
                        tg = lS.tile([128, 256], f16, tag=f"tg{l}")
                        nc.scalar.activation(tg[:], pg[:, 256:512], AF.Tanh,
                                             scale=1 / 64.0)
                        t1 = lS.tile([128, 256], f16, tag=f"t1{l}")
                        nc.vector.tensor_mul(t1[:], sgi[:], tg[:])
                return t1

            def lstm_chain(l, u, t1):
                pg = gate_ps[l]
                # f/o sigmoid: the only post-matmul ACT on the chain
                sfo = lS.tile([128, 512], f16, tag=f"sfo{l}")
                nc.scalar.activation(sfo[:], pg[:, 512:1024], AF.Sigmoid,
                                     scale=1 / 64.0)
                cnew = lS.tile([128, 256], f32, tag=f"c{l}", bufs=2)
                nc.vector.tensor_mul(cnew[:], sfo[:, 0:256], cprev[l][:])
                nc.vector.tensor_add(cnew[:], cnew[:], t1[:])
                cprev[l] = cnew
                tc_ = lS.tile([128, 256], f16, tag=f"tc{l}")
                nc.scalar.activation(tc_[:], cnew[:], AF.Tanh)
                h = lS.tile([128, 256], f16, tag=f"h{l}")
                nc.vector.tensor_mul(h[:], sfo[:, 256:512], tc_[:])
                # PE transpose into a scratch corner of this layer's gates
                # PSUM bank (free between the ACT reads of step u and the
                # matmuls of step u+1) -- costs no extra PSUM banks. The
                # f16 view of the f32 gates tile keeps the transpose at
                # 1 cyc/row.
                pgb = gate_ps[l][:].bitcast(f16)  # (128, 2048)
                for cb in range(2):
                    nc.tensor.transpose(
                        pgb[:, cb * 128:(cb + 1) * 128],
                        h[:, cb * 128:(cb + 1) * 128], eye128_sb[:])
                    hd = hdec[l][:, 0, u, :]  # (128, B) at kc=0
                    dst = bass.AP(hd.tensor, hd.offset + cb * U * B,
                                  [hd.ap[0], [2 * U * B, NG], [1, B]])
                    src_ap = pgb[:, cb * 128:(cb + 1) * 128].rearrange(
                        "p (j r) -> p j r", j=NG)[:, :, 0:B]
                    nc.vector.tensor_copy(dst, src_ap)

            def x1_block(kb):
                hd0 = hdec[0]
                for grp in range(4):
                    x1cw = lS.tile([128, 1024], f16, tag="x1c", bufs=2)
                    for q in range(4):
                        nc2 = grp * 4 + q
                        ps = lP.tile([128, 512], f32, tag="mmps", bufs=2)
                        for kc in range(HK):
                            nc.tensor.matmul(
                                ps[:, 0:256],
                                hd0[:, kc, kb * 16:(kb + 1) * 16, :],
                                wih1_box[0][:, nc2, kc, :],
                                start=(kc == 0), stop=(kc == HK - 1))
                        sl = slice(q * 256, (q + 1) * 256)
                        if with_biases:
                            nc.vector.tensor_add(
                                x1cw[:, sl], ps[:, 0:256],
                                bi1_sb[:, nc2 * 256:(nc2 + 1) * 256])
                        else:
                            nc.vector.tensor_copy(x1cw[:, sl], ps[:, 0:256])
                    nc.gpsimd.dma_start(
                        x1_d.ap()[kb * 4:(kb + 1) * 4, grp, :, :],
                        x1cw[:])

            def zd_chunk(k0, nk):
                # zd for u in [8*k0, 8*(k0+nk)): (J-part, u, b); wdec is
                # SBUF-resident; nk=2 halves the per-u LDWEIGHTS overhead
                zdt = pc.tile([128, JC, 16, B], f16, tag="zd", bufs=2)
                for jc in range(JC):
                    zp = lP.tile([128, 512], f32, tag="mmps", bufs=2)
                    for kc in range(HK):
                        nc.tensor.matmul(
                            zp[:, 0:64 * nk], wdec_sb[:, jc, kc, :],
                            hdec[1][:, kc, 8 * k0:8 * (k0 + nk), :]
                            .rearrange("p u b -> p (u b)"),
                            start=(kc == 0), stop=(kc == HK - 1))
                    nc.vector.tensor_copy(
                        zdt[:, jc, 0:8 * nk, :].rearrange("p u b -> p (u b)"),
                        zp[:, 0:64 * nk])
                return zdt

            def joint_stage1(k, b, zdt, ko):
                # 128 rows = 16 tl pairs (batch b) x 8 u (u in chunk k).
                # per-jc ops: small quanta so the LSTM chain's ACT/DVE ops
                # are not delayed behind a long-running one
                zjt = lS.tile([128, JC, 128], f16, tag="zjt", bufs=2)
                zj = lS.tile([128, JC, 128], f16, tag="zj", bufs=3)
                for jc in range(JC):
                    ze_bc = ze_sb[:, jc, b * TSH:(b + 1) * TSH].to_broadcast(
                        (128, TSH, 8))
                    zdv = zdt[:, jc, 8 * ko:8 * ko + 8, b]  # (128, 8)
                    zd_bc = bass.AP(zdv.tensor, zdv.offset,
                                    [zdv.ap[0], [0, TSH], zdv.ap[1]])
                    nc.vector.tensor_tensor(
                        zjt[:, jc, :].rearrange("p (a u) -> p a u", a=TSH),
                        ze_bc, zd_bc, op=mybir.AluOpType.add)
                    nc.scalar.activation(zj[:, jc, :], zjt[:, jc, :],
                                         AF.Tanh)
                return zj

            def joint_stage2(zj, k, b):
                # jc-outer: each LDWEIGHTS (zj[jc]) is shared by both n2
                # halves and hides under the previous jc's second matmul
                osb = lS.tile([128, OD], f16, tag="osb")
                ops_ = [lP.tile([128, 512], f32, tag="outps", bufs=2,
                                name=f"outps{n2}") for n2 in range(2)]
                for jc in range(JC):
                    for n2 in range(2):
                        nc.tensor.matmul(
                            ops_[n2][:],
                            zj[:, jc, :],
                            wout_sb[:, jc, n2 * 512:(n2 + 1) * 512],
                            start=(jc == 0), stop=(jc == JC - 1))
                for n2 in range(2):
                    if with_out_bias:
                        nc.vector.tensor_add(
                            osb[:, n2 * 512:(n2 + 1) * 512], ops_[n2][:],
                            bout_sb[:, n2 * 512:(n2 + 1) * 512])
                    else:
                        nc.vector.tensor_copy(
                            osb[:, n2 * 512:(n2 + 1) * 512], ops_[n2][:])
                # out rows: b*TSH*U + tl*U + u, tl in [0, 16), u in chunk k
                nc.sync.dma_start(
                    out_d.ap().rearrange("(b tl u) od -> b tl u od",
                                         b=B, tl=TSH)[
                        b, :, 8 * k:8 * (k + 1), :],
                    osb[:])

            # ---- emission: wavefronts with everything interleaved ----
            with nc.named_scope("gat0"):
                gather_chunk(0)
            # whh0 chunked per-kc behind the gather: packs of step u=1 can
            # start after the first ~1MB lands instead of the full 8MB.
            # Spread over gpsimd/sync so the two streams run in parallel.
            for kc in range(HK):
                q = (nc.gpsimd, nc.sync)[kc % 2]
                q.dma_start(whh_sb[0][:, kc], whh0_d.ap()[:, kc])
            with nc.named_scope("x0b0"):
                x0_block(0)
            xq_fetch(0, 0)

            jq = []        # pending joint blocks (k, blk, zd tile, ko)
            pending2 = []  # [(zj, k, blk)] awaiting stage2
            # zd emission: chunk k needs l1 step 8k+7 done (wavefront
            # 24+8k).  Chunks 2..5 are paired (zd matmul efficiency);
            # 0,1,6,7 stay single so early fill starts at w24 and the
            # last chunk still lands at w80 without growing the tail.
            zd_at = {24: [(0, 1)], 32: [(1, 1)], 48: [(2, 2)],
                     64: [(4, 2)], 72: [(6, 1)], 80: [(7, 1)]}
            for w in range(U + 17):
                steps = []
                if w < U:
                    steps.append((0, w))
                if w >= 17:
                    steps.append((1, w - 17))
                # quad X prefetches (l0 quad q at w=4q-1; l1 quad q at
                # w=4q+16, i.e. one l1-step before first use and after the
                # producing x1 burst's emission)
                if w % 4 == 3 and (w + 1) // 4 < U // 4:
                    xq_fetch(0, (w + 1) // 4)
                if w >= 16 and (w - 16) % 4 == 0 and (w - 16) // 4 < U // 4:
                    xq_fetch(1, (w - 16) // 4)
                # both layers' matmul phases first: one contiguous PE
                # burst in the static Tensor-queue order
                with nc.named_scope(f"w{w:02d}"):
                    t1s = {}
                    for l, u in steps:
                        t1s[l] = lstm_mms(l, u)
                # high-priority PE fillers: joint out-matmuls + x1 burst
                for zj, k, blk in pending2:
                    with nc.named_scope(f"jb{k}_{blk}"):
                        joint_stage2(zj, k, blk)
                pending2 = []
                # chain tails LAST among the wavefront's PE work: their
                # transposes rank below the jb fillers so the Tensor queue
                # never stalls on chain-gated work mid-burst
                with nc.named_scope(f"c{w:02d}"):
                    for l, u in steps:
                        lstm_chain(l, u, t1s[l])
                # x1 burst MUST be emitted after this wavefront's scatter
                # (it reads hdec0 up to and including this wavefront's u)
                if w % 16 == 15 and (w - 15) // 16 < UG:
                    with nc.named_scope(f"x1b{(w - 15) // 16}"):
                        x1_block((w - 15) // 16)
                # prologue fillers early on
                if w == 1:
                    nc.sync.dma_start(hst_sb[:], hst_d.ap())
                if 1 <= w <= 3:
                    with nc.named_scope(f"gat{w}"):
                        gather_chunk(w)
                if 4 <= w <= 6 and w - 3 < UG:
                    with nc.named_scope(f"x0b{w - 3}"):
                        x0_block(w - 3)
                if 12 <= w <= 16:
                    with nc.named_scope(f"ze{w - 12}"):
                        ze_jc(w - 12)
                if w == 6:
                    nc.sync.dma_start(wdec_sb[:], wdec_d.ap())
                if w == 8:
                    pre_ctx.__exit__(None, None, None)
                    pw1_ctx = tc.tile_pool(name="whh1p", bufs=1)
                    pw1 = pw1_ctx.__enter__()
                    whh_sb[1] = pw1.tile([128, HK, NG, 1024], f8,
                                         tag="whh1", name="whh1")
                    wih1_sb = pw1.tile([128, 16, HK, 256], f8, tag="wih1",
                                       name="wih1")
                    wih1_box[0] = wih1_sb
                    for kc in range(HK):
                        q = (nc.sync, nc.gpsimd)[kc % 2]
                        q.dma_start(whh_sb[1][:, kc], whh1_d.ap()[:, kc])
                        q2 = (nc.gpsimd, nc.sync)[kc % 2]
                        q2.dma_start(wih1_sb[:, 2 * kc:2 * kc + 2],
                                     wih1_d.ap()[:, 2 * kc:2 * kc + 2])
                if w == 10:
                    nc.gpsimd.dma_start(wout_sb[:, 0:3], wout_d.ap()[:, 0:3])
                    nc.gpsimd.dma_start(wout_sb[:, 3:5], wout_d.ap()[:, 3:5])
                # stage1 of the next joint blocks (low priority this
                # wavefront: their ACT/DVE ops rank below the LSTM chain)
                while jq and len(pending2) < 3:
                    k, blk, zdt, ko = jq.pop(0)
                    with nc.named_scope(f"js{k}_{blk}"):
                        pending2.append(
                            (joint_stage1(k, blk, zdt, ko), k, blk))
                # zd once layer-1 u-sub-block done (lowest priority: its
                # matmuls are pure filler and must not head-of-line block
                # the next wavefront's packs)
                for k0, nk in zd_at.get(w, []):
                    with nc.named_scope(f"zd{k0}"):
                        zdt = zd_chunk(k0, nk)
                    jq.extend((k0 + ko, blk, zdt, ko)
                              for ko in range(nk) for blk in range(8))
            # tail: remaining joint blocks
            while jq or pending2:
                for zj, k, blk in pending2:
                    with nc.named_scope(f"jb{k}_{blk}"):
                        joint_stage2(zj, k, blk)
                pending2 = []
                while jq and len(pending2) < 3:
                    k, blk, zdt, ko = jq.pop(0)
                    with nc.named_scope(f"js{k}_{blk}"):
                        pending2.append(
                            (joint_stage1(k, blk, zdt, ko), k, blk))
            pw1_ctx.__exit__(None, None, None)

    nc.compile()
    return nc


# ---------------- host-side prep ----------------

def gate_perm():
    """perm[j*1024 + s] -> row index in torch (i,f,g,o) 4H gate layout,
    with group-local order [i|g|f|o] (hf0 = i,g; hf1 = f,o)."""
    perm = np.zeros(4 * H, dtype=np.int64)
    for j in range(NG):
        base = j * 1024
        hid = np.arange(256) + j * 256
        perm[base + 0:base + 256] = 0 * H + hid      # i
        perm[base + 256:base + 512] = 2 * H + hid    # g
        perm[base + 512:base + 768] = 1 * H + hid    # f
        perm[base + 768:base + 1024] = 3 * H + hid   # o
    return perm


def prep_inputs(hs_pad, ys_in_pad, embed, W_ih0, W_hh0, b_ih0, b_hh0,
                W_ih1, W_hh1, b_ih1, b_hh1, W_enc, b_enc, W_dec, W_out, b_out,
                U=64, n_cores=8):
    perm = gate_perm()

    def wiht(W, KD, KC):  # (4H, KD) -> (128, KC, 4096) fp16, permuted gates
        Wp = W[perm]                      # (4096, KD)
        return np.ascontiguousarray(
            Wp.T.reshape(KC, 128, 4096).transpose(1, 0, 2)).astype(np.float16)

    def whht(W):  # (4H, H) -> (128, HK, NG, 1024) fp16
        Wp = W[perm]                      # (4096, 1024) rows=permuted gates
        # [p, kc, j, n] = Wp[j*1024+n, kc*128+p]
        a = Wp.T.reshape(HK, 128, NG, 1024).transpose(1, 0, 2, 3)
        return np.ascontiguousarray(a).astype(np.float16)

    ins = {}
    # embed table resident: [p, s, e] = embed[s*128+p, e]
    emb = np.asarray(embed, np.float32).astype(np.float16)
    ins["embedt"] = np.ascontiguousarray(
        emb.reshape(8, 128, E).transpose(1, 0, 2))
    ys = np.asarray(ys_in_pad).astype(np.int64)   # (B, U)
    NCH = B * U // 128
    yy = np.zeros((128, NCH), np.float32)
    for ch in range(NCH):
        p = np.arange(128)
        yy[:, ch] = ys[p % 8, ch * 16 + p // 8]
    ins["yidxr"] = np.ascontiguousarray(
        np.tile(yy.T.reshape(1, B * U), (128, 1)))
    ins["slotv"] = np.ascontiguousarray(
        (np.arange(128)[:, None] + 128.0 * np.arange(8)[None, :])
        .astype(np.float32))
    # LSTM-side weights are fp8(x64); the gate ACTs divide by 64.
    # (numpy-checked: quantizing all four adds ~2.4e-4 output error)
    def f8x(a):
        return (np.asarray(a, np.float32) * 64.0).astype(
            ml_dtypes.float8_e4m3fn)
    # [p, nc, ec, n] = Wp[nc*512+n, ec*128+p] (nc-chunked for split DMA)
    w0 = wiht(W_ih0, E, EK)  # (128, EK, 4096)
    ins["wih0t"] = f8x(np.ascontiguousarray(
        w0.reshape(128, EK, 8, 512).transpose(0, 2, 1, 3)))
    w1 = wiht(W_ih1, H, HK)  # (128, HK, 4096)
    ins["wih1t"] = f8x(np.ascontiguousarray(
        w1.reshape(128, HK, 16, 256).transpose(0, 2, 1, 3)))
    ins["whh0t"] = f8x(whht(W_hh0))
    ins["whh1t"] = f8x(whht(W_hh1))
    ins["eye128"] = np.eye(128, dtype=np.float16)
    # quad-step selector: injq[32j+r, uu, c] = 1 iff r == 8*uu + c
    inj = np.zeros((128, 4, 8), np.float16)
    for j in range(NG):
        for uu in range(4):
            inj[32 * j + 8 * uu:32 * j + 8 * uu + 8, uu] = np.eye(
                8, dtype=np.float16)
    ins["injrep"] = inj
    # [p, ec, jc, m] = W[jc*128+m, ec*128+p]
    def wjt(W, KC):
        a = W.T.reshape(KC, 128, JC, 128).transpose(2, 1, 0, 3)
        return np.ascontiguousarray(a).astype(np.float16)
    ins["wenct"] = wjt(W_enc, HK)
    # wdec resident f16: [p, jc, kc, m] = W[jc*128+m, kc*128+p]
    ins["wdect"] = np.ascontiguousarray(wjt(W_dec, HK).transpose(1, 0, 2, 3))
    # [p, jc, od] = W_out[od, jc*128+p]
    ins["woutt"] = np.ascontiguousarray(
        W_out.T.reshape(JC, 128, OD).transpose(1, 0, 2)).astype(np.float16)
    ins["benc"] = np.ascontiguousarray(
        b_enc.reshape(JC, 128).T).astype(np.float32)
    ins["boutrep"] = np.tile(np.asarray(b_out, np.float32)[None, :], (128, 1))
    ins["bihh0"] = np.tile(
        (64.0 * (b_ih0 + b_hh0)[perm]).astype(np.float16)[None, :], (128, 1))
    ins["bihh1"] = np.tile(
        (64.0 * (b_ih1 + b_hh1)[perm]).astype(np.float16)[None, :], (128, 1))

    maps = []
    for c in range(n_cores):
        m = dict(ins)
        # [p, ec, r] = hs[b, TSH*c + tl, ec*128+p], r = b*TSH+tl
        sl = np.asarray(hs_pad[:, TSH * c:TSH * (c + 1), :], np.float32)
        a = sl.reshape(B * TSH, HK, 128).transpose(2, 1, 0)
        m["hst16"] = np.ascontiguousarray(a).astype(np.float16)
        maps.append(m)
    return maps


def gather_output(results):
    outs = [np.asarray(r["out"], np.float32).reshape(B, TSH, -1, OD)
            for r in results]
    return np.concatenate(outs, axis=1)


# ---------------- entry point ----------------
import sys as _sys
import types as _types

# Recreate the missing antenv.axon_hooks so trace=True works under axon
# (used only when BASS_TRACE=1 is set by a profiling harness).
if "antenv.axon_hooks" not in _sys.modules:
    _m = _types.ModuleType("antenv.axon_hooks")

    def _get_hook():
        try:
            from trn_agent_boot.trn_boot import _ntff_profile_via_ctypes
            return _ntff_profile_via_ctypes("/opt/axon/libaxon_pjrt.so")
        except Exception:
            return None
    _m.get_axon_ntff_profile_hook = _get_hook
    _sys.modules["antenv.axon_hooks"] = _m

_NC = None
last_results = None


def kernel(**inputs):
    """Full-input RNN-T decoder: returns (B, T, U, ODIM) float32."""
    global _NC, last_results
    from concourse.bass_utils import run_bass_kernel_spmd
    U = int(np.asarray(inputs["ys_in_pad"]).shape[1])
    wb = any(float(np.abs(np.asarray(inputs[k])).max()) != 0.0
             for k in ("b_ih0", "b_hh0", "b_ih1", "b_hh1"))
    wob = float(np.abs(np.asarray(inputs["b_out"])).max()) != 0.0
    if _NC is None:
        _NC = build_program(U=U, n_cores=8, with_biases=wb, with_out_bias=wob)
    maps = prep_inputs(**inputs, U=U)
    res = run_bass_kernel_spmd(_NC, maps, core_ids=list(range(8)))
    last_results = res
    return gather_output(res.results)

